# revision 16
# baseline (speedup 1.0000x reference)
"""MinimalMamba Trainium2 kernel — 8-core tensor-parallel over d_inner.

Contract: kernel(**inputs) takes the full unsharded inputs from
reference.setup_inputs() and returns the full (B, S, D_MODEL) output.

Strategy (per core c, d-shard = d_inner/8 = 256 channels):
  - All activations live in [channel, token] layout so every matmul has its
    contraction dim on partitions with naturally-laid-out weights as lhsT.
  - Host pre-transposes x to xT [d_model, B*S] fp16 and slices all weights.
  - Phase 1 is ordered to launch the x_proj AllReduce as early as possible:
    b0 xb-half in_proj -> causal conv (tensor_scalar taps + adds) -> SiLU ->
    x_proj -> CC(0); then batch 1 (combined halves); the z-half of b0 runs
    off the critical path.
  - dt = softplus via Exp then Ln(x+1) on ACT (fused bias).
  - Selective scan per (batch, d-tile, n) slab [128, S]:
      decay = Exp(dt * A[:,n]) on ACT (fp16), u = dtxb * B_bcast (DVE 2x),
      h = tensor_tensor_scan (DVE), hC = h * C_bcast, y = sum_n hC via
      identity-matmul PSUM accumulation; D*xb is folded into the same PSUM
      group via a diag(D) matmul so the gate is one multiply by silu(z).
    B/C rows are partition-broadcast into [128, S] bf16 slabs by DMA.
  - out_proj partials [d_model, B*S] fp16, summed on host in fp32.
"""
import sys

sys.path.insert(0, '/opt/trn_rl_repo')

from contextlib import ExitStack

import numpy as np
import ml_dtypes

import concourse.bass as bass
import concourse.tile as tile
from concourse import bacc, mybir, masks
from concourse.bass_utils import run_bass_kernel_spmd

FP32 = mybir.dt.float32
FP16 = mybir.dt.float16
BF16 = mybir.dt.bfloat16
AF = mybir.ActivationFunctionType
OP = mybir.AluOpType

D_MODEL = 1024
D_STATE = 16
D_CONV = 4
D_INNER = 2048
DT_RANK = 128
BATCH = 2
N_CORES = 8
DSH = D_INNER // N_CORES  # 256 channels per core


def build_nc(S, n_cores=N_CORES):
    T = S                      # tokens per batch
    S2 = BATCH * S             # total tokens
    CH = min(512, T)           # matmul N-chunk
    NCH = T // CH              # chunks per batch
    assert T % CH == 0

    nc = bacc.Bacc("TRN2", target_bir_lowering=False, debug=False,
                   num_devices=n_cores)

    xT_d = nc.dram_tensor("xT", [D_MODEL, S2], FP16, kind="ExternalInput").ap()
    wxz_d = nc.dram_tensor("wxz", [D_MODEL, 2 * DSH], FP16, kind="ExternalInput").ap()
    convw_d = nc.dram_tensor("convw", [DSH, D_CONV], FP32, kind="ExternalInput").ap()
    convb_d = nc.dram_tensor("convb", [DSH, 1], FP32, kind="ExternalInput").ap()
    xpw_d = nc.dram_tensor("xpw", [DSH, DT_RANK + 2 * D_STATE], FP16, kind="ExternalInput").ap()
    dtw_d = nc.dram_tensor("dtw", [DT_RANK, DSH], FP16, kind="ExternalInput").ap()
    dtb_d = nc.dram_tensor("dtb", [DSH, 1], FP32, kind="ExternalInput").ap()
    A_d = nc.dram_tensor("A", [DSH, D_STATE], FP32, kind="ExternalInput").ap()
    Dv_d = nc.dram_tensor("Dv", [DSH, 1], FP32, kind="ExternalInput").ap()
    wo_d = nc.dram_tensor("wo", [DSH, D_MODEL], FP16, kind="ExternalInput").ap()
    outT_d = nc.dram_tensor("outT", [D_MODEL, S2], FP16, kind="ExternalOutput").ap()

    cc_in = [nc.dram_tensor(f"cc_in{b}", [DT_RANK + 2 * D_STATE, T], BF16).ap()
             for b in range(BATCH)]
    cc_out = [nc.dram_tensor(f"cc_out{b}", [DT_RANK + 2 * D_STATE, T], BF16,
                             addr_space="Shared").ap()
              for b in range(BATCH)]

    NK = D_MODEL // 128        # 8 K-tiles for in_proj
    NDT = DSH // 128           # 2 d-tiles per core
    NMO = D_MODEL // 128       # 8 M-tiles for out_proj
    NQ = T // CH               # y-accumulate quarter tiles

    with TileCtx(nc) as (tc, P):
        consts = P("consts", 1)
        xtp = P("xt", 2)
        psA = P("psA", 3, space="PSUM")
        psB = P("psB", 1, space="PSUM")
        psY = P("psY", NQ, space="PSUM")
        actb = P("actb", 1)
        convp = P("convp", 1)
        scanb = P("scan", 2)
        ub = P("ub", 2)
        bcb = P("bc", 2)
        outb = P("outsb", 2)

        # ---- constants ----
        wxz = []
        for k in range(NK):
            t = consts.tile([128, 2 * DSH], FP16, name=f"wxz{k}", tag=f"wxz{k}")
            nc.gpsimd.dma_start(t[:], wxz_d[k * 128:(k + 1) * 128, :])
            wxz.append(t)
        xpw = []
        for j in range(NDT):
            t = consts.tile([128, DT_RANK + 2 * D_STATE], FP16, name=f"xpw{j}", tag=f"xpw{j}")
            nc.gpsimd.dma_start(t[:], xpw_d[j * 128:(j + 1) * 128, :])
            xpw.append(t)
        dtw = consts.tile([128, DSH], FP16, name="dtw", tag="dtw")
        nc.gpsimd.dma_start(dtw[:], dtw_d[:])
        wo = []
        for j in range(NDT):
            t = consts.tile([128, D_MODEL], FP16, name=f"wo{j}", tag=f"wo{j}")
            nc.gpsimd.dma_start(t[:], wo_d[j * 128:(j + 1) * 128, :])
            wo.append(t)
        convw, convb, dtb, Acol, Dv = [], [], [], [], []
        for j in range(NDT):
            for lst, src, w in ((convw, convw_d, D_CONV), (convb, convb_d, 1),
                                (dtb, dtb_d, 1), (Acol, A_d, D_STATE), (Dv, Dv_d, 1)):
                t = consts.tile([128, w], FP32, name=f"c_{j}_{w}_{src.name}",
                                tag=f"c_{j}_{w}_{src.name}")
                nc.gpsimd.dma_start(t[:], src[j * 128:(j + 1) * 128, :])
                lst.append(t)
        ident = consts.tile([128, 128], FP16, name="ident", tag="ident")
        masks.make_identity(nc, ident[:])
        # diag(D) per j-tile: fold the D*xb skip term into the y PSUM group.
        diagD = []
        for j in range(NDT):
            t = consts.tile([128, 128], FP16, name=f"diagD{j}", tag=f"diagD{j}")
            nc.vector.tensor_scalar(t[:], ident[:], Dv[j][:], None, op0=OP.mult)
            diagD.append(t)

        state = [{} for _ in range(BATCH)]

        def inproj_setup(b):
            st = state[b]
            st["xb_pre"] = [actb.tile([128, 3 + T], FP16, name=f"xbpre{j}", tag=f"xbpre{j}", bufs=2) for j in range(NDT)]
            st["zb_s"] = [actb.tile([128, T], FP16, name=f"zbs{j}", tag=f"zbs{j}", bufs=2) for j in range(NDT)]
            for j in range(NDT):
                nc.vector.memset(st["xb_pre"][j][:, 0:3], 0.0)

        def inproj_chunk(b, ch, parts):
            st = state[b]
            tok0 = b * T
            xb_pre, zb_s = st["xb_pre"], st["zb_s"]
            cols = bass.ts(ch, CH)
            xt = [xtp.tile([128, CH], FP16, name=f"xt{k}", tag=f"xt{k}") for k in range(NK)]
            for k in range(NK):
                nc.sync.dma_start(xt[k][:], xT_d[k * 128:(k + 1) * 128,
                                                 tok0 + ch * CH: tok0 + (ch + 1) * CH])
            for j in range(NDT):
                if 0 in parts:
                    ps = psA.tile([128, CH], FP32, name="psA", tag="psA")
                    for k in range(NK):
                        nc.tensor.matmul(ps[:], lhsT=wxz[k][:, j * 128:(j + 1) * 128],
                                         rhs=xt[k][:], start=(k == 0), stop=(k == NK - 1))
                    nc.scalar.copy(xb_pre[j][:, 3 + ch * CH: 3 + (ch + 1) * CH], ps[:])
                if 1 in parts:
                    ps2 = psA.tile([128, CH], FP32, name="psA", tag="psA")
                    for k in range(NK):
                        nc.tensor.matmul(ps2[:], lhsT=wxz[k][:, DSH + j * 128: DSH + (j + 1) * 128],
                                         rhs=xt[k][:], start=(k == 0), stop=(k == NK - 1))
                    nc.scalar.activation(zb_s[j][:, cols], ps2[:], AF.Silu)

        def conv_silu(b):
            st = state[b]
            xb_pre = st["xb_pre"]
            xb_s = [actb.tile([128, T], FP16, name=f"xbs{j}", tag=f"xbs{j}", bufs=2) for j in range(NDT)]
            st["xb_s"] = xb_s
            for j in range(NDT):
                acc = convp.tile([128, T], FP16, name="acc", tag="acc")
                tmp = convp.tile([128, T], FP16, name="tmp", tag="tmp")
                nc.vector.tensor_scalar(acc[:], xb_pre[j][:, 3:3 + T],
                                        convw[j][:, 3:4], convb[j][:],
                                        op0=OP.mult, op1=OP.add)
                for k in range(3):
                    nc.vector.tensor_scalar(tmp[:], xb_pre[j][:, k:k + T],
                                            convw[j][:, k:k + 1], None, op0=OP.mult)
                    nc.vector.tensor_add(acc[:], acc[:], tmp[:])
                nc.scalar.activation(xb_s[j][:], acc[:], AF.Silu)

        def xproj_cc(b):
            st = state[b]
            xb_s = st["xb_s"]
            xd_dt = actb.tile([128, T], BF16, name="xd_dt", tag="xd_dt")
            xd_bcs = actb.tile([32, T], BF16, name="xd_bcs", tag="xd_bcs")
            xd_bc = xd_bcs[:]
            for ch in range(NCH):
                ps = psB.tile([128, CH], FP32, name="psB", tag="psB")
                for j in range(NDT):
                    nc.tensor.matmul(ps[:], lhsT=xpw[j][:, 0:DT_RANK],
                                     rhs=xb_s[j][:, bass.ts(ch, CH)],
                                     start=(j == 0), stop=(j == NDT - 1))
                nc.scalar.copy(xd_dt[:, bass.ts(ch, CH)], ps[:])
                ps2 = psB.tile([32, CH], FP32, name="psB", tag="psB")
                for j in range(NDT):
                    nc.tensor.matmul(ps2[:], lhsT=xpw[j][:, DT_RANK:],
                                     rhs=xb_s[j][:, bass.ts(ch, CH)],
                                     start=(j == 0), stop=(j == NDT - 1))
                nc.scalar.copy(xd_bc[:, bass.ts(ch, CH)], ps2[:])
            nc.sync.dma_start(cc_in[b][0:DT_RANK, :], xd_dt[:])
            nc.sync.dma_start(cc_in[b][DT_RANK:, :], xd_bc[:])
            nc.gpsimd.collective_compute(
                "AllReduce", OP.add,
                replica_groups=[list(range(n_cores))],
                ins=[cc_in[b][:]], outs=[cc_out[b][:]],
            )

        def proj_postCC(b):
            st = state[b]
            xdr16 = actb.tile([128, T], BF16, name="xdr16", tag="xdr16", bufs=2)
            nc.sync.dma_start(xdr16[:], cc_out[b][0:DT_RANK, :])

            dt16 = [actb.tile([128, T], FP16, name=f"dt16_{j}", tag=f"dt16_{j}", bufs=2) for j in range(NDT)]
            dtxb = [actb.tile([128, T], FP16, name=f"dtxb{j}", tag=f"dtxb{j}", bufs=2) for j in range(NDT)]
            for j in range(NDT):
                etile = actb.tile([128, T], FP16, name="etile", tag="etile")
                for ch in range(NCH):
                    ps = psB.tile([128, CH], FP32, name="psB", tag="psB")
                    nc.tensor.matmul(ps[:], lhsT=dtw[:, j * 128:(j + 1) * 128],
                                     rhs=xdr16[:, bass.ts(ch, CH)], start=True, stop=True)
                    nc.scalar.activation(etile[:, bass.ts(ch, CH)], ps[:], AF.Exp,
                                         bias=dtb[j][:])
                nc.scalar.activation(dt16[j][:], etile[:], AF.Ln, bias=1.0)
                nc.vector.tensor_mul(dtxb[j][:], dt16[j][:], st["xb_s"][j][:])
            st["dt16"] = dt16
            st["dtxb"] = dtxb
            st["ygz"] = [actb.tile([128, T], FP16, name=f"ygz{j}", tag=f"ygz{j}", bufs=2) for j in range(NDT)]

        def outproj_piece(b, ch, mo, tail=False):
            tok0 = b * T
            ygz = state[b]["ygz"]
            ps = psA.tile([128, CH], FP32, name="psA", tag="psA")
            for j in range(NDT):
                nc.tensor.matmul(ps[:], lhsT=wo[j][:, mo * 128:(mo + 1) * 128],
                                 rhs=ygz[j][:, bass.ts(ch, CH)],
                                 start=(j == 0), stop=(j == NDT - 1))
            osb = outb.tile([128, CH], FP16, name="osb", tag="osb")
            nc.scalar.copy(osb[:], ps[:])
            nc.sync.dma_start(outT_d[mo * 128:(mo + 1) * 128,
                                     tok0 + ch * CH: tok0 + (ch + 1) * CH], osb[:])

        def scan_j(b, j, extra=None, after_gate=None):
            st = state[b]
            dt16, dtxb, zb_s, ygz = (st["dt16"], st["dtxb"], st["zb_s"], st["ygz"])
            psy = [psY.tile([128, CH], FP32, name="psy", tag="psy") for _ in range(NQ)]
            for q in range(NQ):
                nc.tensor.matmul(psy[q][:], lhsT=diagD[j][:],
                                 rhs=st["xb_s"][j][:, bass.ts(q, CH)],
                                 start=True, stop=False)
            for n in range(D_STATE):
                Bbc = bcb.tile([128, T], BF16, name="Bbc", tag="Bbc")
                nc.gpsimd.dma_start(Bbc[:], cc_out[b][DT_RANK + n:DT_RANK + n + 1, :].partition_broadcast(128))
                Cbc = bcb.tile([128, T], BF16, name="Cbc", tag="Cbc")
                nc.gpsimd.dma_start(Cbc[:], cc_out[b][DT_RANK + D_STATE + n:DT_RANK + D_STATE + n + 1, :].partition_broadcast(128))
                decay = scanb.tile([128, T], FP16, name="decay", tag="decay")
                nc.scalar.activation(decay[:], dt16[j][:], AF.Exp,
                                     scale=Acol[j][:, n:n + 1])
                u = ub.tile([128, T], FP16, name="u", tag="u")
                nc.vector.tensor_mul(u[:], dtxb[j][:], Bbc[:])
                h = scanb.tile([128, T], FP16, name="h", tag="h", bufs=1)
                nc.vector.tensor_tensor_scan(h[:], decay[:], u[:], 0.0,
                                             op0=OP.mult, op1=OP.add)
                hc = scanb.tile([128, T], FP16, name="hc", tag="hc")
                nc.vector.tensor_mul(hc[:], h[:], Cbc[:])
                for q in range(NQ):
                    nc.tensor.matmul(psy[q][:], lhsT=ident[:],
                                     rhs=hc[:, bass.ts(q, CH)],
                                     start=False, stop=(n == D_STATE - 1))
                if extra is not None:
                    extra(n)
            for q in range(NQ):
                nc.vector.tensor_mul(ygz[j][:, bass.ts(q, CH)], psy[q][:],
                                     zb_s[j][:, bass.ts(q, CH)])
                if after_gate is not None:
                    after_gate(q)

        def scan_j_split(b, j, extra=None):
            """Last-scan variant: two chained token-halves so the first two
            quarters gate (and their out_proj pieces start) while the second
            half is still scanning. Reuses the full-size tile tags via views."""
            st = state[b]
            dt16, dtxb, zb_s, ygz = (st["dt16"], st["dtxb"], st["zb_s"], st["ygz"])
            TQ = T // 2
            NQH = TQ // CH
            psy = [psY.tile([128, CH], FP32, name="psy", tag="psy") for _ in range(NQ)]
            for q in range(NQ):
                nc.tensor.matmul(psy[q][:], lhsT=diagD[j][:],
                                 rhs=st["xb_s"][j][:, bass.ts(q, CH)],
                                 start=True, stop=False)
            hlast = actb.tile([128, D_STATE], FP16, name="hlast", tag="hlast")
            for half in range(2):
                lo = half * TQ
                for n in range(D_STATE):
                    Bbc = bcb.tile([128, T], BF16, name="Bbc", tag="Bbc")
                    nc.gpsimd.dma_start(Bbc[:, 0:TQ], cc_out[b][DT_RANK + n:DT_RANK + n + 1, lo:lo + TQ].partition_broadcast(128))
                    Cbc = bcb.tile([128, T], BF16, name="Cbc", tag="Cbc")
                    nc.gpsimd.dma_start(Cbc[:, 0:TQ], cc_out[b][DT_RANK + D_STATE + n:DT_RANK + D_STATE + n + 1, lo:lo + TQ].partition_broadcast(128))
                    decay = scanb.tile([128, T], FP16, name="decay", tag="decay")
                    nc.scalar.activation(decay[:, 0:TQ], dt16[j][:, lo:lo + TQ],
                                         AF.Exp, scale=Acol[j][:, n:n + 1])
                    u = ub.tile([128, T], FP16, name="u", tag="u")
                    nc.vector.tensor_mul(u[:, 0:TQ], dtxb[j][:, lo:lo + TQ], Bbc[:, 0:TQ])
                    h = scanb.tile([128, T], FP16, name="h", tag="h", bufs=1)
                    init = 0.0 if half == 0 else hlast[:, n:n + 1]
                    nc.vector.tensor_tensor_scan(h[:, 0:TQ], decay[:, 0:TQ],
                                                 u[:, 0:TQ], init,
                                                 op0=OP.mult, op1=OP.add)
                    if half == 0:
                        nc.vector.tensor_copy(hlast[:, n:n + 1], h[:, TQ - 1:TQ])
                    hc = scanb.tile([128, T], FP16, name="hc", tag="hc")
                    nc.vector.tensor_mul(hc[:, 0:TQ], h[:, 0:TQ], Cbc[:, 0:TQ])
                    for qq in range(NQH):
                        q = half * NQH + qq
                        nc.tensor.matmul(psy[q][:], lhsT=ident[:],
                                         rhs=hc[:, bass.ts(qq, CH)],
                                         start=False, stop=(n == D_STATE - 1))
                    if extra is not None:
                        extra(half * D_STATE + n)
                for qq in range(NQH):
                    q = half * NQH + qq
                    nc.vector.tensor_mul(ygz[j][:, bass.ts(q, CH)], psy[q][:],
                                         zb_s[j][:, bass.ts(q, CH)])
                    for mo in range(NMO):
                        outproj_piece(b, q, mo)

        # ---- phase schedule ----
        # Critical path to CC(0): b0 xb-half -> conv -> x_proj -> AllReduce.
        inproj_setup(0)
        for ch in range(NCH):
            inproj_chunk(0, ch, (0,))
        conv_silu(0)
        xproj_cc(0)
        # Batch 1: combined halves (xt loaded once), then its CC.
        inproj_setup(1)
        for ch in range(NCH):
            inproj_chunk(1, ch, (0, 1))
        conv_silu(1)
        xproj_cc(1)
        # b0 z-half off the critical path (xt reloaded for these 4 chunks).
        for ch in range(NCH):
            inproj_chunk(0, ch, (1,))

        proj_postCC(0)
        scan_j(0, 0)
        proj_postCC(1)
        scan_j(0, 1)
        pieces = [(ch, mo) for ch in range(NCH) for mo in range(NMO)]
        nhalf = len(pieces) // 2

        def mk_extra(plist):
            it = iter(plist)
            def extra(n):
                try:
                    ch, mo = next(it)
                except StopIteration:
                    return
                outproj_piece(0, ch, mo)
            return extra

        scan_j(1, 0, mk_extra(pieces[:nhalf]))
        scan_j_split(1, 1, mk_extra(pieces[nhalf:]))

    nc.compile()
    return nc


class TileCtx:
    """TileContext + pool ExitStack helper."""
    def __init__(self, nc):
        self.nc = nc
        self.stack = ExitStack()

    def __enter__(self):
        self.tc = tile.TileContext(self.nc)
        self.stack.enter_context(self.tc)

        def P(name, bufs, space="SBUF"):
            return self.stack.enter_context(
                self.tc.tile_pool(name=name, bufs=bufs, space=space))

        return self.tc, P

    def __exit__(self, *a):
        return self.stack.__exit__(*a)


def host_prep(inputs):
    x = np.asarray(inputs["x"], np.float32)
    in_proj_w = np.asarray(inputs["in_proj_w"], np.float32)
    conv_w = np.asarray(inputs["conv_w"], np.float32)      # (4, 1, 2048) WIO
    conv_b = np.asarray(inputs["conv_b"], np.float32)
    x_proj_w = np.asarray(inputs["x_proj_w"], np.float32)
    dt_proj_w = np.asarray(inputs["dt_proj_w"], np.float32)
    dt_proj_b = np.asarray(inputs["dt_proj_b"], np.float32)
    A_log = np.asarray(inputs["A_log"], np.float32)
    Dvec = np.asarray(inputs["D"], np.float32)
    out_proj_w = np.asarray(inputs["out_proj_w"], np.float32)

    S = x.shape[1]
    S2 = BATCH * S
    xT = np.ascontiguousarray(x.reshape(S2, D_MODEL).T).astype(np.float16)
    A = -np.exp(A_log)

    in_maps = []
    for c in range(N_CORES):
        sl = slice(c * DSH, (c + 1) * DSH)
        wxz = np.concatenate([in_proj_w[:, sl],
                              in_proj_w[:, D_INNER + c * DSH: D_INNER + (c + 1) * DSH]],
                             axis=1).astype(np.float16)
        in_maps.append({
            "xT": xT,
            "wxz": np.ascontiguousarray(wxz),
            "convw": np.ascontiguousarray(conv_w[:, 0, sl].T).astype(np.float32),
            "convb": conv_b[sl].reshape(DSH, 1).astype(np.float32),
            "xpw": np.ascontiguousarray(x_proj_w[sl, :]).astype(np.float16),
            "dtw": np.ascontiguousarray(dt_proj_w[:, sl]).astype(np.float16),
            "dtb": dt_proj_b[sl].reshape(DSH, 1).astype(np.float32),
            "A": np.ascontiguousarray(A[sl, :]).astype(np.float32),
            "Dv": Dvec[sl].reshape(DSH, 1).astype(np.float32),
            "wo": np.ascontiguousarray(out_proj_w[sl, :]).astype(np.float16),
        })
    return in_maps


_NC_CACHE = {}


def get_nc(S):
    if S not in _NC_CACHE:
        _NC_CACHE[S] = build_nc(S)
    return _NC_CACHE[S]


def run(inputs, trace=False):
    S = np.asarray(inputs["x"]).shape[1]
    nc = get_nc(S)
    in_maps = host_prep(inputs)
    res = run_bass_kernel_spmd(nc, in_maps, list(range(N_CORES)), trace=trace)
    S2 = BATCH * S
    outT = np.zeros((D_MODEL, S2), np.float32)
    for c in range(N_CORES):
        outT += res.results[c]["outT"].astype(np.float32)
    out = outT.T.reshape(BATCH, S, D_MODEL)
    return out, res


def kernel(**inputs):
    out, _ = run(inputs)
    return out


# revision 17
# speedup vs baseline: 1.0126x; 1.0126x over previous
"""MinimalMamba Trainium2 kernel — 8-core tensor-parallel over d_inner.

Contract: kernel(**inputs) takes the full unsharded inputs from
reference.setup_inputs() and returns the full (B, S, D_MODEL) output.

Strategy (per core c, d-shard = d_inner/8 = 256 channels):
  - All activations live in [channel, token] layout so every matmul has its
    contraction dim on partitions with naturally-laid-out weights as lhsT.
  - Host pre-transposes x to xT [d_model, B*S] fp16 and slices all weights.
  - Phase 1 is ordered to launch the x_proj AllReduce as early as possible:
    b0 xb-half in_proj -> causal conv (tensor_scalar taps + adds) -> SiLU ->
    x_proj -> CC(0); then batch 1 (combined halves); the z-half of b0 runs
    off the critical path.
  - dt = softplus via Exp then Ln(x+1) on ACT (fused bias).
  - Selective scan per (batch, d-tile, n) slab [128, S]:
      decay = Exp(dt * A[:,n]) on ACT (fp16), u = dtxb * B_bcast (DVE 2x),
      h = tensor_tensor_scan (DVE), hC = h * C_bcast, y = sum_n hC via
      identity-matmul PSUM accumulation; D*xb is folded into the same PSUM
      group via a diag(D) matmul so the gate is one multiply by silu(z).
    B/C rows are partition-broadcast into [128, S] bf16 slabs by DMA.
  - out_proj partials [d_model, B*S] fp16, summed on host in fp32.
"""
import sys

sys.path.insert(0, '/opt/trn_rl_repo')

from contextlib import ExitStack

import numpy as np
import ml_dtypes

import concourse.bass as bass
import concourse.tile as tile
from concourse import bacc, mybir, masks
from concourse.bass_utils import run_bass_kernel_spmd

FP32 = mybir.dt.float32
FP16 = mybir.dt.float16
BF16 = mybir.dt.bfloat16
AF = mybir.ActivationFunctionType
OP = mybir.AluOpType

D_MODEL = 1024
D_STATE = 16
D_CONV = 4
D_INNER = 2048
DT_RANK = 128
BATCH = 2
N_CORES = 8
DSH = D_INNER // N_CORES  # 256 channels per core


def build_nc(S, n_cores=N_CORES):
    T = S                      # tokens per batch
    S2 = BATCH * S             # total tokens
    CH = min(512, T)           # matmul N-chunk
    NCH = T // CH              # chunks per batch
    assert T % CH == 0

    nc = bacc.Bacc("TRN2", target_bir_lowering=False, debug=False,
                   num_devices=n_cores)

    xT_d = nc.dram_tensor("xT", [D_MODEL, S2], FP16, kind="ExternalInput").ap()
    wxz_d = nc.dram_tensor("wxz", [D_MODEL, 2 * DSH], FP16, kind="ExternalInput").ap()
    convw_d = nc.dram_tensor("convw", [DSH, D_CONV], FP32, kind="ExternalInput").ap()
    convb_d = nc.dram_tensor("convb", [DSH, 1], FP32, kind="ExternalInput").ap()
    xpw_d = nc.dram_tensor("xpw", [DSH, DT_RANK + 2 * D_STATE], FP16, kind="ExternalInput").ap()
    dtw_d = nc.dram_tensor("dtw", [DT_RANK, DSH], FP16, kind="ExternalInput").ap()
    dtb_d = nc.dram_tensor("dtb", [DSH, 1], FP32, kind="ExternalInput").ap()
    A_d = nc.dram_tensor("A", [DSH, D_STATE], FP32, kind="ExternalInput").ap()
    Dv_d = nc.dram_tensor("Dv", [DSH, 1], FP32, kind="ExternalInput").ap()
    wo_d = nc.dram_tensor("wo", [DSH, D_MODEL], FP16, kind="ExternalInput").ap()
    outT_d = nc.dram_tensor("outT", [D_MODEL, S2], FP16, kind="ExternalOutput").ap()

    cc_in = [nc.dram_tensor(f"cc_in{b}", [DT_RANK + 2 * D_STATE, T], BF16).ap()
             for b in range(BATCH)]
    cc_out = [nc.dram_tensor(f"cc_out{b}", [DT_RANK + 2 * D_STATE, T], BF16,
                             addr_space="Shared").ap()
              for b in range(BATCH)]

    NK = D_MODEL // 128        # 8 K-tiles for in_proj
    NDT = DSH // 128           # 2 d-tiles per core
    NMO = D_MODEL // 128       # 8 M-tiles for out_proj
    NQ = T // CH               # y-accumulate quarter tiles

    with TileCtx(nc) as (tc, P):
        consts = P("consts", 1)
        xtp = P("xt", 2)
        psA = P("psA", 3, space="PSUM")
        psB = P("psB", 1, space="PSUM")
        psY = P("psY", NQ, space="PSUM")
        actb = P("actb", 1)
        convp = P("convp", 1)
        scanb = P("scan", 2)
        ub = P("ub", 2)
        bcb = P("bc", 2)
        outb = P("outsb", 2)

        # ---- constants ----
        wxz = []
        for k in range(NK):
            t = consts.tile([128, 2 * DSH], FP16, name=f"wxz{k}", tag=f"wxz{k}")
            nc.gpsimd.dma_start(t[:], wxz_d[k * 128:(k + 1) * 128, :])
            wxz.append(t)
        xpw = []
        for j in range(NDT):
            t = consts.tile([128, DT_RANK + 2 * D_STATE], FP16, name=f"xpw{j}", tag=f"xpw{j}")
            nc.gpsimd.dma_start(t[:], xpw_d[j * 128:(j + 1) * 128, :])
            xpw.append(t)
        dtw = consts.tile([128, DSH], FP16, name="dtw", tag="dtw")
        nc.gpsimd.dma_start(dtw[:], dtw_d[:])
        wo = []
        for j in range(NDT):
            t = consts.tile([128, D_MODEL], FP16, name=f"wo{j}", tag=f"wo{j}")
            nc.gpsimd.dma_start(t[:], wo_d[j * 128:(j + 1) * 128, :])
            wo.append(t)
        convw, convb, dtb, Acol, Dv = [], [], [], [], []
        for j in range(NDT):
            for lst, src, w in ((convw, convw_d, D_CONV), (convb, convb_d, 1),
                                (dtb, dtb_d, 1), (Acol, A_d, D_STATE), (Dv, Dv_d, 1)):
                t = consts.tile([128, w], FP32, name=f"c_{j}_{w}_{src.name}",
                                tag=f"c_{j}_{w}_{src.name}")
                nc.gpsimd.dma_start(t[:], src[j * 128:(j + 1) * 128, :])
                lst.append(t)
        ident = consts.tile([128, 128], FP16, name="ident", tag="ident")
        masks.make_identity(nc, ident[:])
        # diag(D) per j-tile: fold the D*xb skip term into the y PSUM group.
        diagD = []
        for j in range(NDT):
            t = consts.tile([128, 128], FP16, name=f"diagD{j}", tag=f"diagD{j}")
            nc.vector.tensor_scalar(t[:], ident[:], Dv[j][:], None, op0=OP.mult)
            diagD.append(t)

        state = [{} for _ in range(BATCH)]

        def inproj_setup(b):
            st = state[b]
            st["xb_pre"] = [actb.tile([128, 3 + T], FP16, name=f"xbpre{j}", tag=f"xbpre{j}", bufs=2) for j in range(NDT)]
            st["zb_s"] = [actb.tile([128, T], FP16, name=f"zbs{j}", tag=f"zbs{j}", bufs=2) for j in range(NDT)]
            st["xb_s"] = [actb.tile([128, T], FP16, name=f"xbs{j}", tag=f"xbs{j}", bufs=2) for j in range(NDT)]
            for j in range(NDT):
                nc.vector.memset(st["xb_pre"][j][:, 0:3], 0.0)

        def inproj_chunk(b, ch, parts):
            st = state[b]
            tok0 = b * T
            xb_pre, zb_s = st["xb_pre"], st["zb_s"]
            cols = bass.ts(ch, CH)
            xt = [xtp.tile([128, CH], FP16, name=f"xt{k}", tag=f"xt{k}") for k in range(NK)]
            for k in range(NK):
                nc.sync.dma_start(xt[k][:], xT_d[k * 128:(k + 1) * 128,
                                                 tok0 + ch * CH: tok0 + (ch + 1) * CH])
            for j in range(NDT):
                if 0 in parts:
                    ps = psA.tile([128, CH], FP32, name="psA", tag="psA")
                    for k in range(NK):
                        nc.tensor.matmul(ps[:], lhsT=wxz[k][:, j * 128:(j + 1) * 128],
                                         rhs=xt[k][:], start=(k == 0), stop=(k == NK - 1))
                    nc.scalar.copy(xb_pre[j][:, 3 + ch * CH: 3 + (ch + 1) * CH], ps[:])
                if 1 in parts:
                    ps2 = psA.tile([128, CH], FP32, name="psA", tag="psA")
                    for k in range(NK):
                        nc.tensor.matmul(ps2[:], lhsT=wxz[k][:, DSH + j * 128: DSH + (j + 1) * 128],
                                         rhs=xt[k][:], start=(k == 0), stop=(k == NK - 1))
                    nc.scalar.activation(zb_s[j][:, cols], ps2[:], AF.Silu)

        def conv_chunk(b, ch):
            st = state[b]
            xb_pre, xb_s = st["xb_pre"], st["xb_s"]
            lo = ch * CH
            for j in range(NDT):
                acc = convp.tile([128, CH], FP16, name="acc", tag="acc")
                tmp = convp.tile([128, CH], FP16, name="tmp", tag="tmp")
                nc.vector.tensor_scalar(acc[:], xb_pre[j][:, 3 + lo:3 + lo + CH],
                                        convw[j][:, 3:4], convb[j][:],
                                        op0=OP.mult, op1=OP.add)
                for k in range(3):
                    nc.vector.tensor_scalar(tmp[:], xb_pre[j][:, k + lo:k + lo + CH],
                                            convw[j][:, k:k + 1], None, op0=OP.mult)
                    nc.vector.tensor_add(acc[:], acc[:], tmp[:])
                nc.scalar.activation(xb_s[j][:, lo:lo + CH], acc[:], AF.Silu)

        def xproj_chunk(b, ch):
            st = state[b]
            xb_s = st["xb_s"]
            if ch == 0:
                st["xd_dt"] = actb.tile([128, T], BF16, name="xd_dt", tag="xd_dt")
                st["xd_bc"] = actb.tile([32, T], BF16, name="xd_bcs", tag="xd_bcs")
            xd_dt, xd_bc = st["xd_dt"], st["xd_bc"]
            ps = psB.tile([128, CH], FP32, name="psB", tag="psB")
            for j in range(NDT):
                nc.tensor.matmul(ps[:], lhsT=xpw[j][:, 0:DT_RANK],
                                 rhs=xb_s[j][:, bass.ts(ch, CH)],
                                 start=(j == 0), stop=(j == NDT - 1))
            nc.scalar.copy(xd_dt[:, bass.ts(ch, CH)], ps[:])
            ps2 = psB.tile([32, CH], FP32, name="psB", tag="psB")
            for j in range(NDT):
                nc.tensor.matmul(ps2[:], lhsT=xpw[j][:, DT_RANK:],
                                 rhs=xb_s[j][:, bass.ts(ch, CH)],
                                 start=(j == 0), stop=(j == NDT - 1))
            nc.scalar.copy(xd_bc[:, bass.ts(ch, CH)], ps2[:])

        def cc_kick(b):
            st = state[b]
            nc.sync.dma_start(cc_in[b][0:DT_RANK, :], st["xd_dt"][:])
            nc.sync.dma_start(cc_in[b][DT_RANK:, :], st["xd_bc"][:])
            nc.gpsimd.collective_compute(
                "AllReduce", OP.add,
                replica_groups=[list(range(n_cores))],
                ins=[cc_in[b][:]], outs=[cc_out[b][:]],
            )

        def proj_postCC(b):
            st = state[b]
            xdr16 = actb.tile([128, T], BF16, name="xdr16", tag="xdr16", bufs=2)
            nc.sync.dma_start(xdr16[:], cc_out[b][0:DT_RANK, :])

            dt16 = [actb.tile([128, T], FP16, name=f"dt16_{j}", tag=f"dt16_{j}", bufs=2) for j in range(NDT)]
            dtxb = [actb.tile([128, T], FP16, name=f"dtxb{j}", tag=f"dtxb{j}", bufs=2) for j in range(NDT)]
            for j in range(NDT):
                etile = actb.tile([128, T], FP16, name="etile", tag="etile")
                for ch in range(NCH):
                    ps = psB.tile([128, CH], FP32, name="psB", tag="psB")
                    nc.tensor.matmul(ps[:], lhsT=dtw[:, j * 128:(j + 1) * 128],
                                     rhs=xdr16[:, bass.ts(ch, CH)], start=True, stop=True)
                    nc.scalar.activation(etile[:, bass.ts(ch, CH)], ps[:], AF.Exp,
                                         bias=dtb[j][:])
                nc.scalar.activation(dt16[j][:], etile[:], AF.Ln, bias=1.0)
                nc.vector.tensor_mul(dtxb[j][:], dt16[j][:], st["xb_s"][j][:])
            st["dt16"] = dt16
            st["dtxb"] = dtxb
            st["ygz"] = [actb.tile([128, T], FP16, name=f"ygz{j}", tag=f"ygz{j}", bufs=2) for j in range(NDT)]

        def outproj_piece(b, ch, mo, tail=False):
            tok0 = b * T
            ygz = state[b]["ygz"]
            ps = psA.tile([128, CH], FP32, name="psA", tag="psA")
            for j in range(NDT):
                nc.tensor.matmul(ps[:], lhsT=wo[j][:, mo * 128:(mo + 1) * 128],
                                 rhs=ygz[j][:, bass.ts(ch, CH)],
                                 start=(j == 0), stop=(j == NDT - 1))
            osb = outb.tile([128, CH], FP16, name="osb", tag="osb")
            nc.scalar.copy(osb[:], ps[:])
            nc.sync.dma_start(outT_d[mo * 128:(mo + 1) * 128,
                                     tok0 + ch * CH: tok0 + (ch + 1) * CH], osb[:])

        def scan_j(b, j, extra=None, after_gate=None):
            st = state[b]
            dt16, dtxb, zb_s, ygz = (st["dt16"], st["dtxb"], st["zb_s"], st["ygz"])
            psy = [psY.tile([128, CH], FP32, name="psy", tag="psy") for _ in range(NQ)]
            for q in range(NQ):
                nc.tensor.matmul(psy[q][:], lhsT=diagD[j][:],
                                 rhs=st["xb_s"][j][:, bass.ts(q, CH)],
                                 start=True, stop=False)
            for n in range(D_STATE):
                Bbc = bcb.tile([128, T], BF16, name="Bbc", tag="Bbc")
                nc.gpsimd.dma_start(Bbc[:], cc_out[b][DT_RANK + n:DT_RANK + n + 1, :].partition_broadcast(128))
                Cbc = bcb.tile([128, T], BF16, name="Cbc", tag="Cbc")
                nc.gpsimd.dma_start(Cbc[:], cc_out[b][DT_RANK + D_STATE + n:DT_RANK + D_STATE + n + 1, :].partition_broadcast(128))
                decay = scanb.tile([128, T], FP16, name="decay", tag="decay")
                nc.scalar.activation(decay[:], dt16[j][:], AF.Exp,
                                     scale=Acol[j][:, n:n + 1])
                u = ub.tile([128, T], FP16, name="u", tag="u")
                nc.vector.tensor_mul(u[:], dtxb[j][:], Bbc[:])
                h = scanb.tile([128, T], FP16, name="h", tag="h", bufs=1)
                nc.vector.tensor_tensor_scan(h[:], decay[:], u[:], 0.0,
                                             op0=OP.mult, op1=OP.add)
                hc = scanb.tile([128, T], FP16, name="hc", tag="hc")
                nc.vector.tensor_mul(hc[:], h[:], Cbc[:])
                for q in range(NQ):
                    nc.tensor.matmul(psy[q][:], lhsT=ident[:],
                                     rhs=hc[:, bass.ts(q, CH)],
                                     start=False, stop=(n == D_STATE - 1))
                if extra is not None:
                    extra(n)
            for q in range(NQ):
                nc.vector.tensor_mul(ygz[j][:, bass.ts(q, CH)], psy[q][:],
                                     zb_s[j][:, bass.ts(q, CH)])
                if after_gate is not None:
                    after_gate(q)

        def scan_j_split(b, j, extra=None):
            """Last-scan variant: two chained token-halves so the first two
            quarters gate (and their out_proj pieces start) while the second
            half is still scanning. Reuses the full-size tile tags via views."""
            st = state[b]
            dt16, dtxb, zb_s, ygz = (st["dt16"], st["dtxb"], st["zb_s"], st["ygz"])
            TQ = T // 2
            NQH = TQ // CH
            psy = [psY.tile([128, CH], FP32, name="psy", tag="psy") for _ in range(NQ)]
            for q in range(NQ):
                nc.tensor.matmul(psy[q][:], lhsT=diagD[j][:],
                                 rhs=st["xb_s"][j][:, bass.ts(q, CH)],
                                 start=True, stop=False)
            hlast = actb.tile([128, D_STATE], FP16, name="hlast", tag="hlast")
            for half in range(2):
                lo = half * TQ
                for n in range(D_STATE):
                    Bbc = bcb.tile([128, T], BF16, name="Bbc", tag="Bbc")
                    nc.gpsimd.dma_start(Bbc[:, 0:TQ], cc_out[b][DT_RANK + n:DT_RANK + n + 1, lo:lo + TQ].partition_broadcast(128))
                    Cbc = bcb.tile([128, T], BF16, name="Cbc", tag="Cbc")
                    nc.gpsimd.dma_start(Cbc[:, 0:TQ], cc_out[b][DT_RANK + D_STATE + n:DT_RANK + D_STATE + n + 1, lo:lo + TQ].partition_broadcast(128))
                    decay = scanb.tile([128, T], FP16, name="decay", tag="decay")
                    nc.scalar.activation(decay[:, 0:TQ], dt16[j][:, lo:lo + TQ],
                                         AF.Exp, scale=Acol[j][:, n:n + 1])
                    u = ub.tile([128, T], FP16, name="u", tag="u")
                    nc.vector.tensor_mul(u[:, 0:TQ], dtxb[j][:, lo:lo + TQ], Bbc[:, 0:TQ])
                    h = scanb.tile([128, T], FP16, name="h", tag="h", bufs=1)
                    init = 0.0 if half == 0 else hlast[:, n:n + 1]
                    nc.vector.tensor_tensor_scan(h[:, 0:TQ], decay[:, 0:TQ],
                                                 u[:, 0:TQ], init,
                                                 op0=OP.mult, op1=OP.add)
                    if half == 0:
                        nc.vector.tensor_copy(hlast[:, n:n + 1], h[:, TQ - 1:TQ])
                    hc = scanb.tile([128, T], FP16, name="hc", tag="hc")
                    nc.vector.tensor_mul(hc[:, 0:TQ], h[:, 0:TQ], Cbc[:, 0:TQ])
                    for qq in range(NQH):
                        q = half * NQH + qq
                        nc.tensor.matmul(psy[q][:], lhsT=ident[:],
                                         rhs=hc[:, bass.ts(qq, CH)],
                                         start=False, stop=(n == D_STATE - 1))
                    if extra is not None:
                        extra(half * D_STATE + n)
                for qq in range(NQH):
                    q = half * NQH + qq
                    nc.vector.tensor_mul(ygz[j][:, bass.ts(q, CH)], psy[q][:],
                                         zb_s[j][:, bass.ts(q, CH)])
                    for mo in range(NMO):
                        outproj_piece(b, q, mo)

        # ---- phase schedule ----
        # Critical path to CC(0): b0 xb-half -> conv -> x_proj -> AllReduce.
        inproj_setup(0)
        for ch in range(NCH):
            inproj_chunk(0, ch, (0,))
            conv_chunk(0, ch)
            xproj_chunk(0, ch)
        cc_kick(0)
        # Batch 1: combined halves (xt loaded once), then its CC.
        inproj_setup(1)
        for ch in range(NCH):
            inproj_chunk(1, ch, (0, 1))
            conv_chunk(1, ch)
            xproj_chunk(1, ch)
        cc_kick(1)
        # b0 z-half off the critical path (xt reloaded for these 4 chunks).
        for ch in range(NCH):
            inproj_chunk(0, ch, (1,))

        proj_postCC(0)
        scan_j(0, 0)
        proj_postCC(1)
        scan_j(0, 1)
        pieces = [(ch, mo) for ch in range(NCH) for mo in range(NMO)]
        nhalf = len(pieces) // 2

        def mk_extra(plist):
            it = iter(plist)
            def extra(n):
                try:
                    ch, mo = next(it)
                except StopIteration:
                    return
                outproj_piece(0, ch, mo)
            return extra

        scan_j(1, 0, mk_extra(pieces[:nhalf]))
        scan_j_split(1, 1, mk_extra(pieces[nhalf:]))

    nc.compile()
    return nc


class TileCtx:
    """TileContext + pool ExitStack helper."""
    def __init__(self, nc):
        self.nc = nc
        self.stack = ExitStack()

    def __enter__(self):
        self.tc = tile.TileContext(self.nc)
        self.stack.enter_context(self.tc)

        def P(name, bufs, space="SBUF"):
            return self.stack.enter_context(
                self.tc.tile_pool(name=name, bufs=bufs, space=space))

        return self.tc, P

    def __exit__(self, *a):
        return self.stack.__exit__(*a)


def host_prep(inputs):
    x = np.asarray(inputs["x"], np.float32)
    in_proj_w = np.asarray(inputs["in_proj_w"], np.float32)
    conv_w = np.asarray(inputs["conv_w"], np.float32)      # (4, 1, 2048) WIO
    conv_b = np.asarray(inputs["conv_b"], np.float32)
    x_proj_w = np.asarray(inputs["x_proj_w"], np.float32)
    dt_proj_w = np.asarray(inputs["dt_proj_w"], np.float32)
    dt_proj_b = np.asarray(inputs["dt_proj_b"], np.float32)
    A_log = np.asarray(inputs["A_log"], np.float32)
    Dvec = np.asarray(inputs["D"], np.float32)
    out_proj_w = np.asarray(inputs["out_proj_w"], np.float32)

    S = x.shape[1]
    S2 = BATCH * S
    xT = np.ascontiguousarray(x.reshape(S2, D_MODEL).T).astype(np.float16)
    A = -np.exp(A_log)

    in_maps = []
    for c in range(N_CORES):
        sl = slice(c * DSH, (c + 1) * DSH)
        wxz = np.concatenate([in_proj_w[:, sl],
                              in_proj_w[:, D_INNER + c * DSH: D_INNER + (c + 1) * DSH]],
                             axis=1).astype(np.float16)
        in_maps.append({
            "xT": xT,
            "wxz": np.ascontiguousarray(wxz),
            "convw": np.ascontiguousarray(conv_w[:, 0, sl].T).astype(np.float32),
            "convb": conv_b[sl].reshape(DSH, 1).astype(np.float32),
            "xpw": np.ascontiguousarray(x_proj_w[sl, :]).astype(np.float16),
            "dtw": np.ascontiguousarray(dt_proj_w[:, sl]).astype(np.float16),
            "dtb": dt_proj_b[sl].reshape(DSH, 1).astype(np.float32),
            "A": np.ascontiguousarray(A[sl, :]).astype(np.float32),
            "Dv": Dvec[sl].reshape(DSH, 1).astype(np.float32),
            "wo": np.ascontiguousarray(out_proj_w[sl, :]).astype(np.float16),
        })
    return in_maps


_NC_CACHE = {}


def get_nc(S):
    if S not in _NC_CACHE:
        _NC_CACHE[S] = build_nc(S)
    return _NC_CACHE[S]


def run(inputs, trace=False):
    S = np.asarray(inputs["x"]).shape[1]
    nc = get_nc(S)
    in_maps = host_prep(inputs)
    res = run_bass_kernel_spmd(nc, in_maps, list(range(N_CORES)), trace=trace)
    S2 = BATCH * S
    outT = np.zeros((D_MODEL, S2), np.float32)
    for c in range(N_CORES):
        outT += res.results[c]["outT"].astype(np.float32)
    out = outT.T.reshape(BATCH, S, D_MODEL)
    return out, res


def kernel(**inputs):
    out, _ = run(inputs)
    return out
